# revision 1
# baseline (speedup 1.0000x reference)
"""CosineAttention on 8 TRN2 NeuronCores.

Sharding (head + tensor parallel, per the hint):
  core c owns head h=c for both batches:
    - computes qT,kT = [Wq_h|Wk_h]^T-stationary matmuls over full xT
    - RMS-normalizes q,k in the [d, i] layout via a PE ones-matmul
      partition-reduction + K=2 outer-product broadcast
    - simT[j,i] = kn^T qn (K=64, float32r), exp folded scale=1/8 on ACT
    - attn@v with a ones column appended to v so the softmax denominator
      falls out as matmul row 64; normalize by its reciprocal
    - per-batch AllGather of o_cT [64, n] (bf16) -> [512, n] feature-major
    - column-parallel out-proj: outT_c [64, n] = W2_c^T-stationary matmul
  host concatenates the 8 feature slices.

Matmul dtype: float32r (TF32-class single-pass PE mode) for the fp32 path;
bf16 for the post-softmax path (attn weights / v / out-proj operands).
"""

import numpy as np
import ml_dtypes

import concourse.bass as bass
import concourse.tile as tile
from concourse import bacc
import concourse.mybir as mybir
from concourse import bass_utils

f32 = mybir.dt.float32
f32r = mybir.dt.float32r
bf16 = mybir.dt.bfloat16
AF = mybir.ActivationFunctionType

N_CORES = 8
HEADS = 8
D = 64            # head dim
B = 2             # batch
SEQ = 2048        # tokens per batch
DIM = 512         # model dim = HEADS * D
NTOK = B * SEQ    # 4096
EPS = 1e-4
SCALE = D ** -0.5  # 0.125

FT = DIM // 128   # 4 f-tiles of 128
CH1 = 512         # stage-1 token chunk
NCH1 = NTOK // CH1            # 8
ICH = 1024        # phase-2 i-chunk (exp batching)
NICH = SEQ // ICH             # 2 per batch
JT = SEQ // 128   # 16 j-tiles per batch
PCH = 512         # phase-3 chunk
NPCH = SEQ // PCH             # 4 per batch

_BUILD_CACHE = {}


def build(collective=True, num_devices=N_CORES, reps=1):
    key = (collective, num_devices, reps)
    if key in _BUILD_CACHE:
        return _BUILD_CACHE[key]
    nc = bacc.Bacc("TRN2", target_bir_lowering=False, debug=False,
                   num_devices=num_devices)
    xT = nc.dram_tensor("xT", [DIM, NTOK], f32, kind="ExternalInput").ap()
    wqk = nc.dram_tensor("wqk", [DIM, 128], f32, kind="ExternalInput").ap()
    wv = nc.dram_tensor("wv", [DIM, D], f32, kind="ExternalInput").ap()
    w2 = nc.dram_tensor("w2", [DIM, D], bf16, kind="ExternalInput").ap()
    ones2 = nc.dram_tensor("ones2", [128, 2], f32, kind="ExternalInput").ap()
    e2 = nc.dram_tensor("e2", [2, 128], f32, kind="ExternalInput").ap()
    ones1 = nc.dram_tensor("ones1", [1, D], f32, kind="ExternalInput").ap()
    outT = nc.dram_tensor("outT", [D, NTOK], f32, kind="ExternalOutput").ap()

    with tile.TileContext(nc) as tc:
        with (
            tc.tile_pool(name="persist", bufs=1) as pp,
            tc.tile_pool(name="sb", bufs=2) as sb,
            tc.tile_pool(name="ps", bufs=1, space="PSUM") as ps,
            tc.tile_pool(name="dram", bufs=1, space="DRAM") as dram,
            nc.allow_low_precision(reason="f32r matmuls; bf16 attn/out path"),
        ):
            # ---- persistent weights / constants ----
            wqk_sb = pp.tile([128, FT, 128], f32r)
            wv_sb = pp.tile([128, FT, D], f32r)
            w2_sb = pp.tile([128, FT, D], bf16)
            for t in range(FT):
                nc.sync.dma_start(wqk_sb[:, t, :],
                                  wqk[t * 128:(t + 1) * 128, :].bitcast(f32r))
                nc.sync.dma_start(wv_sb[:, t, :],
                                  wv[t * 128:(t + 1) * 128, :].bitcast(f32r))
                nc.sync.dma_start(w2_sb[:, t, :], w2[t * 128:(t + 1) * 128, :])
            o2_sb = pp.tile([128, 2], f32r)
            nc.sync.dma_start(o2_sb[:], ones2[:].bitcast(f32r))
            e2_sb = pp.tile([2, 128], f32r)
            nc.sync.dma_start(e2_sb[:], e2[:].bitcast(f32r))
            o1_sb = pp.tile([1, D], f32r)
            nc.sync.dma_start(o1_sb[:], ones1[:].bitcast(f32r))

            # ---- persistent activations ----
            qn_sb = pp.tile([D, NTOK], f32r)     # normalized qT
            kn_sb = pp.tile([D, NTOK], f32r)     # normalized kT
            qk_all = pp.tile([128, NTOK], f32r)  # raw [q;k]T
            vo_sb = pp.tile([128, NTOK // 128, D + 1], bf16)  # v | ones

            # ---- stage 1: qkv projections + rms normalization ----
            for rep in range(reps):
              for ci in range(NCH1):
                  cols = slice(ci * CH1, (ci + 1) * CH1)
                  xt_sb = sb.tile([128, FT, CH1], f32r, tag="xt")
                  for t in range(FT):
                      nc.sync.dma_start(
                          xt_sb[:, t, :],
                          xT[t * 128:(t + 1) * 128, cols].bitcast(f32r))

                  # qkT chunk: [q;k] x-stream, W stationary
                  qk_ps = ps.tile([128, CH1], f32, tag="small", bufs=4)
                  for t in range(FT):
                      nc.tensor.matmul(qk_ps[:], wqk_sb[:, t, :], xt_sb[:, t, :],
                                       start=(t == 0), stop=(t == FT - 1))
                  # v chunk: [j, d], xT stationary
                  for js in range(CH1 // 128):
                      v_ps = ps.tile([128, D], f32, tag="small", bufs=4)
                      for t in range(FT):
                          nc.tensor.matmul(
                              v_ps[:],
                              xt_sb[:, t, js * 128:(js + 1) * 128],
                              wv_sb[:, t, :],
                              start=(t == 0), stop=(t == FT - 1))
                      jt = ci * (CH1 // 128) + js
                      nc.vector.tensor_copy(vo_sb[:, jt, 0:D], v_ps[:])
                      nc.gpsimd.memset(vo_sb[:, jt, D:D + 1], 1.0)

                  # raw qk to sbuf (ACT; DVE is busier)
                  nc.scalar.activation(qk_all[:, cols], qk_ps[:], AF.Copy)
                  # sq = qk^2 (DVE, from the sbuf copy)
                  sq_sb = sb.tile([128, CH1], f32r, tag="sq")
                  nc.vector.tensor_mul(sq_sb[:], qk_all[:, cols],
                                       qk_all[:, cols])
                  # st[2, CH1] = column sums of q-sq and k-sq
                  st_ps = ps.tile([2, CH1], f32, tag="small", bufs=4)
                  nc.tensor.matmul(st_ps[:], o2_sb[:], sq_sb[:],
                                   start=True, stop=True)
                  # r = 1/(sqrt(st/64) + eps)
                  rt_sb = sb.tile([2, CH1], f32, tag="rt")
                  nc.scalar.activation(rt_sb[:], st_ps[:], AF.Sqrt,
                                       scale=1.0 / D)
                  re_sb = sb.tile([2, CH1], f32, tag="re")
                  nc.vector.tensor_scalar_add(re_sb[:], rt_sb[:], EPS)
                  rc_sb = sb.tile([2, CH1], f32r, tag="rc")
                  nc.vector.reciprocal(rc_sb[:], re_sb[:])
                  # R[128, CH1] = outer(e2, r): row broadcast of scales
                  r_ps = ps.tile([128, CH1], f32, tag="small", bufs=4)
                  nc.tensor.matmul(r_ps[:], e2_sb[:], rc_sb[:],
                                   start=True, stop=True)
                  rb_sb = sb.tile([128, CH1], f32r, tag="rb")
                  nc.scalar.activation(rb_sb[:], r_ps[:], AF.Copy)
                  # apply
                  nc.vector.tensor_mul(qn_sb[:, cols], qk_all[0:D, cols],
                                       rb_sb[0:D, :])
                  nc.vector.tensor_mul(kn_sb[:, cols], qk_all[D:128, cols],
                                       rb_sb[D:128, :])

              # ---- per-batch: attention + allgather + out-proj ----
              cc_outs = []
              for b in range(B):
                  cc_in = dram.tile([D, SEQ], bf16, name=f"cc_in{b}")
                  cc_out = dram.tile([DIM, SEQ], bf16, addr_space="Shared",
                                     name=f"cc_out{b}")
                  cc_outs.append(cc_out)
                  for ic in range(NICH):
                      i0 = b * SEQ + ic * ICH
                      expT = sb.tile([128, JT, ICH], bf16, tag="expT")
                      for jt in range(JT):
                          j0 = b * SEQ + jt * 128
                          sim_ps = ps.tile([128, ICH], f32, tag="big", bufs=2)
                          for h in range(ICH // 512):
                              nc.tensor.matmul(
                                  sim_ps[:, h * 512:(h + 1) * 512],
                                  kn_sb[:, j0:j0 + 128],
                                  qn_sb[:, i0 + h * 512:i0 + (h + 1) * 512],
                                  start=True, stop=True)
                          nc.scalar.activation(expT[:, jt, :], sim_ps[:],
                                               AF.Exp, scale=SCALE)
                      for h in range(ICH // 512):
                          av_ps = ps.tile([D + 1, 512], f32, tag="small", bufs=4)
                          for jt in range(JT):
                              nc.tensor.matmul(
                                  av_ps[:],
                                  vo_sb[:, b * JT + jt, :],
                                  expT[:, jt, h * 512:(h + 1) * 512],
                                  start=(jt == 0), stop=(jt == JT - 1))
                          # normalize by sumexp (row D) and emit bf16
                          rse_sb = sb.tile([1, 512], f32r, tag="rse")
                          nc.vector.reciprocal(rse_sb[:],
                                               av_ps[D:D + 1, :].bitcast(f32r))
                          r2_ps = ps.tile([D, 512], f32, tag="small", bufs=4)
                          nc.tensor.matmul(r2_ps[:], o1_sb[:], rse_sb[:],
                                           start=True, stop=True)
                          r2_sb = sb.tile([D, 512], f32, tag="r2")
                          nc.scalar.activation(r2_sb[:], r2_ps[:], AF.Copy)
                          oc_sb = sb.tile([D, 512], bf16, tag="oc")
                          nc.vector.tensor_mul(oc_sb[:], av_ps[0:D, :], r2_sb[:])
                          nc.sync.dma_start(
                              cc_in[:, ic * ICH + h * 512:
                                    ic * ICH + (h + 1) * 512], oc_sb[:])
                  if collective:
                      nc.gpsimd.collective_compute(
                          "AllGather", mybir.AluOpType.bypass,
                          replica_groups=[list(range(num_devices))],
                          ins=[cc_in[:]], outs=[cc_out[:]])
                  else:
                      # timing-only stand-in: keep the DRAM write traffic
                      nc.sync.dma_start(cc_out[0:D, :], cc_in[:])

              for b in range(B):
                  cc_out = cc_outs[b]
                  for pc in range(NPCH):
                      cols = slice(pc * PCH, (pc + 1) * PCH)
                      ag_sb = sb.tile([128, FT, PCH], bf16, tag="ag")
                      for t in range(FT):
                          nc.sync.dma_start(ag_sb[:, t, :],
                                            cc_out[t * 128:(t + 1) * 128, cols])
                      fp_ps = ps.tile([D, PCH], f32, tag="small", bufs=4)
                      for t in range(FT):
                          nc.tensor.matmul(fp_ps[:], w2_sb[:, t, :],
                                           ag_sb[:, t, :],
                                           start=(t == 0), stop=(t == FT - 1))
                      fo_sb = sb.tile([D, PCH], f32, tag="fo")
                      nc.vector.tensor_copy(fo_sb[:], fp_ps[:])
                      nc.sync.dma_start(
                          outT[:, b * SEQ + pc * PCH:b * SEQ + (pc + 1) * PCH],
                          fo_sb[:])
    nc.compile()
    _BUILD_CACHE[key] = nc
    return nc


def make_in_maps(x, Wq, Wkv, Wout):
    xT = np.ascontiguousarray(x.reshape(NTOK, DIM).T).astype(np.float32)
    ones2 = np.zeros((128, 2), np.float32)
    ones2[0:D, 0] = 1.0
    ones2[D:128, 1] = 1.0
    e2 = np.ascontiguousarray(ones2.T)
    ones1 = np.ones((1, D), np.float32)
    in_maps = []
    for c in range(N_CORES):
        rows = slice(c * D, (c + 1) * D)
        wqk = np.ascontiguousarray(
            np.concatenate([Wq[rows, :].T, Wkv[rows, :].T], axis=1))
        wv = np.ascontiguousarray(Wkv[DIM + c * D:DIM + (c + 1) * D, :].T)
        w2 = np.ascontiguousarray(Wout[rows, :].T).astype(ml_dtypes.bfloat16)
        in_maps.append({
            "xT": xT, "wqk": wqk.astype(np.float32),
            "wv": wv.astype(np.float32), "w2": w2,
            "ones2": ones2, "e2": e2, "ones1": ones1,
        })
    return in_maps


def kernel(x, Wq, Wkv, Wout, _trace=False):
    nc = build()
    in_maps = make_in_maps(np.asarray(x), np.asarray(Wq), np.asarray(Wkv),
                           np.asarray(Wout))
    res = bass_utils.run_bass_kernel_spmd(
        nc, in_maps, core_ids=list(range(N_CORES)), trace=_trace)
    out = np.empty((NTOK, DIM), np.float32)
    for c in range(N_CORES):
        out[:, c * D:(c + 1) * D] = res.results[c]["outT"].T
    full = out.reshape(B, SEQ, DIM)
    if _trace:
        return full, res
    return full



# revision 20
# speedup vs baseline: 1.3481x; 1.3481x over previous
"""CosineAttention on 8 TRN2 NeuronCores — v3 (batch-sequential).

Sharding (head-parallel): core c owns head h=c for both batches.

Per-core pipeline (all-bf16 datapath, fp32 PSUM accum):
  stage 0: load full xT [512, 4096] bf16 into SBUF, DMAs spread over
    two queues so the first chunk lands fast.
  stage 1 (4 chunks of 1024 tokens):
    - qkT = wqk^T-stationary matmuls; sq = qk^2 on ACT (Square, PSUM src)
    - st = per-token sum of squares via ones-matmul
    - r' = rsqrt(st*scl) via ACT Abs_reciprocal_sqrt; scl folds the /64
      mean and the 1/8 attention scale (into the q row only)
    - r' broadcast across partitions via an e2 outer-product matmul,
      ACT-copied to SBUF; DVE muls read qk straight from PSUM
    - qn lands on partitions 0-63; kn is DMA-shifted from partitions
      64-127 down to 0-63 (DVE lanes cannot cross partitions)
    - v: wv-stationary vT matmuls + XBAR DMA transpose into [j, d]
  stage 2, batches SEQUENTIAL (av needs only 2 PSUM banks at a time,
  which frees 6 banks = 3 rotating sim tiles so the PE can run ahead
  and ACT exp — the true bottleneck — stays saturated):
    for b: for ic (i-chunks of 1024): for jt (16 j-tiles):
      simT -> ACT Exp -> expT bf16; AV matmuls pipelined one jt behind
    finalize per (b, ic): 1/se via DVE reciprocal_approx_fast (se copied
    to partition 0 first), GPSIMD partition broadcast, DVE mul -> oc
    AllGather per (b, ic) [64, 1024] bf16 so batch 0's collectives and
    out-projection overlap batch 1's attention; only the last AG tails.
  out-proj per (b, ic): w2^T-stationary matmuls on gathered [512, 1024].
"""

import numpy as np
import ml_dtypes

import concourse.bass as bass
import concourse.tile as tile
from concourse import bacc
import concourse.mybir as mybir
from concourse import bass_utils

f32 = mybir.dt.float32
f32r = mybir.dt.float32r
bf16 = mybir.dt.bfloat16
AF = mybir.ActivationFunctionType

N_CORES = 8
HEADS = 8
D = 64            # head dim
B = 2             # batch
SEQ = 2048        # tokens per batch
DIM = 512         # model dim
NTOK = B * SEQ    # 4096
SCALE = D ** -0.5

FT = DIM // 128   # 4 feature tiles
CH = 1024         # stage-1 token chunk
NCH = NTOK // CH  # 4
IC = 1024         # stage-2 i-chunk
NIC = SEQ // IC   # 2
JT = SEQ // 128   # 16 j-tiles per batch

DEBUG_DUMP = False

_BUILD_CACHE = {}


def _emit_outproj(nc, sb, ps, cc_out, w2_sb, outT, b, ic):
    ag_sb = sb.tile([128, FT, IC], bf16, tag="ag", name=f"ag{b}_{ic}")
    for t in range(FT):
        nc.sync.dma_start(ag_sb[:, t, :], cc_out[t * 128:(t + 1) * 128, :])
    fp_ps = ps.tile([D, IC], f32, tag="sim", bufs=3, name=f"fp{b}_{ic}")
    for h in range(IC // 512):
        hc = slice(h * 512, (h + 1) * 512)
        for t in range(FT):
            nc.tensor.matmul(fp_ps[:, hc], w2_sb[:, t, :], ag_sb[:, t, hc],
                             start=(t == 0), stop=(t == FT - 1))
    fo_sb = sb.tile([D, IC], f32, tag="fo", name=f"fo{b}_{ic}")
    nc.vector.tensor_copy(fo_sb[:], fp_ps[:])
    nc.sync.dma_start(
        outT[:, b * SEQ + ic * IC:b * SEQ + (ic + 1) * IC], fo_sb[:])


def build(num_devices=N_CORES):
    key = (num_devices,)
    if key in _BUILD_CACHE:
        return _BUILD_CACHE[key]
    nc = bacc.Bacc("TRN2", target_bir_lowering=False, debug=False,
                   num_devices=num_devices)
    xT = nc.dram_tensor("xT", [DIM, NTOK], bf16, kind="ExternalInput").ap()
    wqk = nc.dram_tensor("wqk", [DIM, 128], bf16, kind="ExternalInput").ap()
    wv = nc.dram_tensor("wv", [DIM, D], bf16, kind="ExternalInput").ap()
    w2 = nc.dram_tensor("w2", [DIM, D], bf16, kind="ExternalInput").ap()
    o2 = nc.dram_tensor("o2", [128, 2], bf16, kind="ExternalInput").ap()
    scl = nc.dram_tensor("scl", [2, 1], f32, kind="ExternalInput").ap()
    e2 = nc.dram_tensor("e2", [2, 128], f32, kind="ExternalInput").ap()
    outT = nc.dram_tensor("outT", [D, NTOK], f32, kind="ExternalOutput").ap()
    if DEBUG_DUMP:
        dbg_qn = nc.dram_tensor("dbg_qn", [D, NTOK], f32,
                                kind="ExternalOutput").ap()
        dbg_kn = nc.dram_tensor("dbg_kn", [D, NTOK], f32,
                                kind="ExternalOutput").ap()
        dbg_vo = nc.dram_tensor("dbg_vo", [128, 2 * JT * (D + 1)], f32,
                                kind="ExternalOutput").ap()
        dbg_ex = nc.dram_tensor("dbg_ex", [128, IC], f32,
                                kind="ExternalOutput").ap()
        dbg_av = nc.dram_tensor("dbg_av", [D + 1, IC], f32,
                                kind="ExternalOutput").ap()

    with tile.TileContext(nc) as tc:
        with (
            tc.tile_pool(name="persist", bufs=1) as pp,
            tc.tile_pool(name="sb", bufs=2) as sb,
            tc.tile_pool(name="ps", bufs=1, space="PSUM") as ps,
            tc.tile_pool(name="dram", bufs=1, space="DRAM") as dram,
            nc.allow_low_precision(reason="bf16 datapath; approx reciprocal"),
        ):
            # ---- persistent weights / constants ----
            wqk_sb = pp.tile([128, FT, 128], bf16)
            wv_sb = pp.tile([128, FT, D], bf16)
            w2_sb = pp.tile([128, FT, D], bf16)
            for t in range(FT):
                rows = slice(t * 128, (t + 1) * 128)
                nc.scalar.dma_start(wqk_sb[:, t, :], wqk[rows, :])
                nc.scalar.dma_start(wv_sb[:, t, :], wv[rows, :])
                nc.scalar.dma_start(w2_sb[:, t, :], w2[rows, :])
            o2_sb = pp.tile([128, 2], bf16)
            nc.scalar.dma_start(o2_sb[:], o2[:])
            scl_sb = pp.tile([2, 1], f32)
            nc.scalar.dma_start(scl_sb[:], scl[:])
            e2_sb = pp.tile([2, 128], f32r)
            nc.scalar.dma_start(e2_sb[:], e2[:].bitcast(f32r))

            # ---- full xT resident in SBUF; two DMA queues, chunk-major ----
            xt_sb = pp.tile([128, FT, NTOK], bf16)
            for c in range(NCH):
                cols = slice(c * CH, (c + 1) * CH)
                for t in range(FT):
                    rows = slice(t * 128, (t + 1) * 128)
                    eng = nc.sync if t % 2 == 0 else nc.gpsimd
                    eng.dma_start(xt_sb[:, t, cols], xT[rows, cols])

            qn_sb = pp.tile([D, NTOK], bf16)
            kn_sb = pp.tile([D, NTOK], bf16)
            vo_sb = pp.tile([128, 2 * JT, D + 1], bf16)  # v | ones
            nc.gpsimd.memset(vo_sb[:, :, D:D + 1], 1.0)

            # ---- stage 1: projections + cosine normalization ----
            for c in range(NCH):
                qk_ps = ps.tile([128, CH], f32, tag="sim", bufs=3)
                for h in range(CH // 512):
                    hc = slice(h * 512, (h + 1) * 512)
                    xcol = slice(c * CH + h * 512, c * CH + (h + 1) * 512)
                    for t in range(FT):
                        nc.tensor.matmul(qk_ps[:, hc], wqk_sb[:, t, :],
                                         xt_sb[:, t, xcol],
                                         start=(t == 0), stop=(t == FT - 1))
                sq_sb = sb.tile([128, CH], bf16, tag="sq")
                nc.scalar.activation(sq_sb[:], qk_ps[:], AF.Square)
                st_ps = ps.tile([2, CH], f32, tag="av", bufs=1)
                for h in range(CH // 512):
                    hc = slice(h * 512, (h + 1) * 512)
                    nc.tensor.matmul(st_ps[:, hc], o2_sb[:], sq_sb[:, hc],
                                     start=True, stop=True)
                # r'[0] = rsqrt(st_q) = rsqrt(ms_q)/8, r'[1] = rsqrt(ms_k)
                rt_sb = sb.tile([2, CH], f32r, tag="rt")
                nc.scalar.activation(rt_sb[:], st_ps[:],
                                     AF.Abs_reciprocal_sqrt, scale=scl_sb[:])
                r_ps = ps.tile([128, CH], f32, tag="sim", bufs=3)
                for h in range(CH // 512):
                    hc = slice(h * 512, (h + 1) * 512)
                    nc.tensor.matmul(r_ps[:, hc], e2_sb[:], rt_sb[:, hc],
                                     start=True, stop=True)
                rb_sb = sb.tile([128, CH], f32, tag="rb")
                nc.scalar.activation(rb_sb[:], r_ps[:], AF.Copy)
                dcol = slice(c * CH, (c + 1) * CH)
                nc.vector.tensor_mul(qn_sb[:, dcol], qk_ps[0:64, :],
                                     rb_sb[0:64, :])
                kh_sb = sb.tile([128, CH], bf16, tag="kh")
                nc.vector.tensor_mul(kh_sb[64:128, :], qk_ps[64:128, :],
                                     rb_sb[64:128, :])
                nc.scalar.dma_start(kn_sb[:, dcol], kh_sb[64:128, :])

                # v: wv-stationary vT then XBAR DMA transpose into [j, d]
                vt_ps = ps.tile([D, CH], f32, tag="av", bufs=1)
                for h in range(CH // 512):
                    hc = slice(h * 512, (h + 1) * 512)
                    xcol = slice(c * CH + h * 512, c * CH + (h + 1) * 512)
                    for t in range(FT):
                        nc.tensor.matmul(vt_ps[:, hc], wv_sb[:, t, :],
                                         xt_sb[:, t, xcol],
                                         start=(t == 0), stop=(t == FT - 1))
                vt_sb = sb.tile([D, CH], bf16, tag="vt")
                nc.vector.tensor_copy(vt_sb[:], vt_ps[:])
                vj_sb = sb.tile([128, CH // 128, D], bf16, tag="vj")
                nc.sync.dma_start_transpose(vj_sb[:], vt_sb[:])
                nc.vector.tensor_copy(vo_sb[:, c * 8:(c + 1) * 8, 0:D],
                                      vj_sb[:])

            if DEBUG_DUMP:
                for nm, t_sb, dst in (("qn", qn_sb, dbg_qn),
                                      ("kn", kn_sb, dbg_kn)):
                    d_f = sb.tile([D, NTOK], f32, tag=f"dbg{nm}", bufs=1)
                    nc.vector.tensor_copy(d_f[:], t_sb[:])
                    nc.sync.dma_start(dst[:], d_f[:])
                dv = sb.tile([128, 2 * JT, D + 1], f32, tag="dbgv", bufs=1)
                nc.vector.tensor_copy(dv[:], vo_sb[:])
                nc.sync.dma_start(dbg_vo[:], dv[:])

            # ---- stage 2: attention, batches sequential ----
            b0_cc = None
            for b in range(B):
                cc_pairs = []
                for ic in range(NIC):
                    cc_in = dram.tile([D, IC], bf16, name=f"cc_in{b}_{ic}")
                    cc_out = dram.tile([DIM, IC], bf16, addr_space="Shared",
                                       name=f"cc_out{b}_{ic}")
                    cc_pairs.append((cc_in, cc_out))
                    av = ps.tile([D + 1, IC], f32, tag="av", bufs=1,
                                 name=f"av{b}_{ic}")
                    ex_prev = None
                    for jt in range(JT + 1):
                        ex_cur = None
                        if jt < JT:
                            sim_ps = ps.tile([128, IC], f32, tag="sim",
                                             bufs=3, name="sim")
                            for h in range(IC // 512):
                                hc = slice(h * 512, (h + 1) * 512)
                                nc.tensor.matmul(
                                    sim_ps[:, hc],
                                    kn_sb[:, b * SEQ + jt * 128:
                                          b * SEQ + (jt + 1) * 128],
                                    qn_sb[:, b * SEQ + ic * IC + h * 512:
                                          b * SEQ + ic * IC + (h + 1) * 512],
                                    start=True, stop=True)
                            ex_cur = sb.tile([128, IC], bf16, tag="ex",
                                             bufs=6, name="ex")
                            nc.scalar.activation(ex_cur[:], sim_ps[:], AF.Exp)
                            if DEBUG_DUMP and b == 0 and ic == 0 and jt == 0:
                                de = sb.tile([128, IC], f32, tag="dbge",
                                             bufs=1)
                                nc.vector.tensor_copy(de[:], ex_cur[:])
                                nc.sync.dma_start(dbg_ex[:], de[:])
                        if jt > 0:
                            pjt = jt - 1
                            for h in range(IC // 512):
                                hc = slice(h * 512, (h + 1) * 512)
                                nc.tensor.matmul(av[:, hc],
                                                 vo_sb[:, b * JT + pjt, :],
                                                 ex_prev[:, hc],
                                                 start=(pjt == 0),
                                                 stop=(pjt == JT - 1))
                        ex_prev = ex_cur
                        # overlap batch-0 out-projection with batch-1
                        # attention once its AllGathers have landed
                        if b == 1 and ic == 1 and jt == 6:
                            _emit_outproj(nc, sb, ps, b0_cc[0][1], w2_sb,
                                          outT, 0, 0)
                        if b == 1 and ic == 1 and jt == 10:
                            _emit_outproj(nc, sb, ps, b0_cc[1][1], w2_sb,
                                          outT, 0, 1)

                    if DEBUG_DUMP and b == 0 and ic == 0:
                        da = sb.tile([D + 1, IC], f32, tag="dbga", bufs=1)
                        nc.vector.tensor_copy(da[:], av[:])
                        nc.sync.dma_start(dbg_av[:], da[:])
                    # finalize: oc = av * (1/se)
                    se0_sb = sb.tile([1, IC], f32, tag="se0")
                    nc.vector.tensor_copy(se0_sb[:], av[D:D + 1, :])
                    rse_sb = sb.tile([1, IC], f32, tag="rse")
                    nc.vector.reciprocal_approx_fast(rse_sb[:], se0_sb[:])
                    rb2_sb = sb.tile([D, IC], f32, tag="rb2")
                    nc.gpsimd.partition_broadcast(rb2_sb[:], rse_sb[:])
                    oc_sb = sb.tile([D, IC], bf16, tag="oc")
                    nc.vector.tensor_mul(oc_sb[:], av[0:D, :], rb2_sb[:])
                    nc.sync.dma_start(cc_in[:], oc_sb[:])
                    nc.gpsimd.collective_compute(
                        "AllGather", mybir.AluOpType.bypass,
                        replica_groups=[list(range(num_devices))],
                        ins=[cc_in[:]], outs=[cc_out[:]])
                if b == 0:
                    b0_cc = cc_pairs

            # ---- out-projection for batch 1 (batch 0 was interleaved) ----
            for ic in range(NIC):
                _emit_outproj(nc, sb, ps, cc_pairs[ic][1], w2_sb, outT, 1, ic)
    nc.compile()
    _BUILD_CACHE[key] = nc
    return nc


def make_in_maps(x, Wq, Wkv, Wout):
    to_bf = lambda a: np.ascontiguousarray(a).astype(ml_dtypes.bfloat16)
    xT = to_bf(x.reshape(NTOK, DIM).T)
    o2 = np.zeros((128, 2), np.float32)
    o2[0:D, 0] = 1.0
    o2[D:128, 1] = 1.0
    o2 = o2.astype(ml_dtypes.bfloat16)
    scl = np.array([[1.0], [1.0 / D]], np.float32)
    e2 = np.zeros((2, 128), np.float32)
    e2[0, 0:D] = 1.0
    e2[1, D:128] = 1.0
    in_maps = []
    for c in range(N_CORES):
        rows = slice(c * D, (c + 1) * D)
        wqk = to_bf(np.concatenate([Wq[rows, :].T, Wkv[rows, :].T], axis=1))
        wv = to_bf(Wkv[DIM + c * D:DIM + (c + 1) * D, :].T)
        w2 = to_bf(Wout[rows, :].T)
        in_maps.append({
            "xT": xT, "wqk": wqk, "wv": wv, "w2": w2, "o2": o2, "scl": scl,
            "e2": e2,
        })
    return in_maps


def kernel(x, Wq, Wkv, Wout, _trace=False):
    nc = build()
    in_maps = make_in_maps(np.asarray(x), np.asarray(Wq), np.asarray(Wkv),
                           np.asarray(Wout))
    res = bass_utils.run_bass_kernel_spmd(
        nc, in_maps, core_ids=list(range(N_CORES)), trace=_trace)
    out = np.empty((NTOK, DIM), np.float32)
    for c in range(N_CORES):
        out[:, c * D:(c + 1) * D] = res.results[c]["outT"].T
    full = out.reshape(B, SEQ, DIM)
    if _trace:
        return full, res
    return full
